# revision 10
# baseline (speedup 1.0000x reference)
"""MultiHeadAttention (no-softmax, causal -inf mask) Bass kernel for 8 TRN2 cores.

Sharding: core c -> batch b=c//2, head group g=c%2 (8 of 16 heads).
Each core: QKV projection for its heads, causal attention, partial output
projection over its heads' rows of w2. Host sums the two partials per batch
and adds b2.

Layouts (per core, all fp32):
  xt    [1024, 2048]  x[b].T
  w1kq  [1024, 1024]  [w1 k-cols | 0.125*w1 q-cols] for the 8 heads
  b1kq  [128, 8]      per-Mtile bias columns for the kq projection
  w1v   [1024, 512]   w1 v-cols
  b1v   [1, 512]      v bias row
  w2s   [512, 1024]   w2 rows for the 8 heads
  ut    [2048, 16]    UT[k, jj] = -inf if k >= 128*(jj+1) else 0
  mdiag [128, 128]    mdiag[k, q] = -inf if k > q else 0
  out   [2048, 1024]  partial (x[b] attention output) @ w2s
"""
import numpy as np

import concourse.bass as bass
import concourse.mybir as mybir
import concourse.tile as tile
from concourse import bacc
from concourse.bass_utils import run_bass_kernel_spmd

F32 = mybir.dt.float32
T = 2048
D = 1024
H = 16
DH = 64
NCORES = 8
NEG_INF = np.float32(-np.inf)


def build_body(nc, tc, tensors):
    xt, w1kq, b1kq, w1v, b1v, w2s, ut, mdiag, out = tensors
    KT = D // 128      # 8 contraction tiles for projections
    MT_KQ = 8          # 1024 kq cols -> 8 Mtiles
    MT_T = T // 128    # 16
    NCH = T // 512     # 4

    import contextlib
    ctx = contextlib.ExitStack()
    with ctx:
        const = ctx.enter_context(tc.tile_pool(name="const", bufs=1))
        persist = ctx.enter_context(tc.tile_pool(name="persist", bufs=1))

        # ---- constants / persistent tiles ----
        ut_sb = const.tile([128, 16, 16], F32)
        nc.sync.dma_start(ut_sb[:], ut[:].rearrange("(kt p) j -> p kt j", p=128))
        mdiag_sb = const.tile([128, 128], F32)
        nc.sync.dma_start(mdiag_sb[:], mdiag[:])
        b1kq_sb = const.tile([128, 8], F32)
        nc.sync.dma_start(b1kq_sb[:], b1kq[:])
        b1v_sb = const.tile([1, 512], F32)
        nc.sync.dma_start(b1v_sb[:], b1v[:])
        ones_sb = const.tile([1, 512], F32)
        nc.gpsimd.memset(ones_sb[:], 1.0)
        w2s_sb = const.tile([128, 4, 1024], F32)
        nc.sync.dma_start(w2s_sb[:], w2s[:].rearrange("(kt p) j -> p kt j", p=128))

        ykq = persist.tile([128, 8, T], F32)      # [cols, T] k rows 0..511, q 512..1023 (by Mtile)
        v_sb = persist.tile([128, 16, 512], F32)  # [T, vcols]
        outT = persist.tile([128, 4, T], F32)     # attention out^T, 2 heads per pair row-block
        infpT_sb = persist.tile([128, 4, 16], F32)  # -inf suffix contribution [headcol, pair, qtile]

        # ================= Stage B: kq projection =================
        # ykq[c, t] = sum_d w1kq[d, c] * xt[d, t] + b1kq[c]
        with (
            tc.tile_pool(name="s1", bufs=1) as s1,
            tc.tile_pool(name="s1w", bufs=2) as s1w,
            tc.tile_pool(name="ps1", bufs=4, space="PSUM") as ps1,
        ):
            xt3 = xt[:].rearrange("(kt p) t -> p kt t", p=128)
            for nch in range(NCH):
                xtc = {}
                for kt in range(KT):
                    xtc[kt] = s1.tile([128, 512], F32, tag="xtc", name=f"xtc{kt}", bufs=9)
                    nc.sync.dma_start(xtc[kt][:], xt3[:, kt, 512 * nch:512 * (nch + 1)])
                for m in range(MT_KQ):
                    wk = s1w.tile([128, 8, 128], F32, tag="wkq")
                    nc.sync.dma_start(
                        wk[:], w1kq[:].rearrange("(kt p) c -> p kt c", p=128)[:, :, 128 * m:128 * (m + 1)])
                    ps = ps1.tile([128, 512], F32)
                    for kt in range(KT):
                        nc.tensor.matmul(ps[:], wk[:, kt, :], xtc[kt][:],
                                         start=(kt == 0), stop=(kt == KT - 1))
                    # PSUM -> SBUF with per-partition bias (ACT)
                    nc.scalar.add(ykq[:, m, 512 * nch:512 * (nch + 1)], ps[:],
                                  b1kq_sb[:, m:m + 1])

            # ================= Stage C: v projection =================
            # v[t, c] = sum_d xt[d, t] * w1v[d, c] + b1v[c]
            w1v_sb = s1.tile([128, 8, 512], F32, tag="w1v", bufs=1)
            nc.sync.dma_start(w1v_sb[:], w1v[:].rearrange("(kt p) c -> p kt c", p=128))
            for mt in range(MT_T):
                xl = s1w.tile([128, 8, 128], F32, tag="xlt")
                nc.sync.dma_start(xl[:], xt3[:, :, 128 * mt:128 * (mt + 1)])
                ps = ps1.tile([128, 512], F32)
                for kt in range(KT):
                    nc.tensor.matmul(ps[:], xl[:, kt, :], w1v_sb[:, kt, :],
                                     start=(kt == 0), stop=False)
                # bias row via K=1 matmul: v += ones.T @ b1v
                nc.tensor.matmul(ps[:], ones_sb[0:1, 0:128], b1v_sb[0:1, :],
                                 start=False, stop=True)
                nc.vector.tensor_copy(v_sb[:, mt, :], ps[:])

            # ================= Stage D: inf-suffix table =================
            # infpT[c, jj] = sum_k v[k, c] * ut[k, jj]   (IEEE -inf semantics)
            for p4 in range(4):
                psd = ps1.tile([128, 16], F32, tag="psd", name=f"psd{p4}", bufs=2)
                for kt in range(MT_T):
                    nc.tensor.matmul(psd[:], v_sb[:, kt, 128 * p4:128 * (p4 + 1)],
                                     ut_sb[:, kt, :],
                                     start=(kt == 0), stop=(kt == MT_T - 1))
                nc.vector.tensor_copy(infpT_sb[:, p4, :], psd[:])

        # ================= Stage E: attention =================
        with (
            tc.tile_pool(name="st", bufs=3) as stp,
            tc.tile_pool(name="pse", bufs=2, space="PSUM") as pse,
            tc.tile_pool(name="pso", bufs=2, space="PSUM") as pso,
        ):
            copy_flip = 0
            for p in range(4):
                kblk = ykq[:, p, :]       # [128, T] two heads' k (rows 0:64 / 64:128)
                qblk = ykq[:, 4 + p, :]
                for qh in range(2):
                    qbase = 1024 * qh
                    imax = 8 if qh == 0 else 16
                    ops = pso.tile([128, 1024], F32, tag="ops")
                    for i in range(imax):
                        qlo = max(128 * i, qbase)
                        # chunks aligned to absolute 512 boundaries
                        c0 = 512 * (qlo // 512)
                        for cs in range(c0, qbase + 1024, 512):
                            lo = max(qlo, cs)
                            hi = cs + 512
                            N = hi - lo
                            psA = pse.tile([128, 512], F32, tag="psA")
                            psB = pse.tile([128, 512], F32, tag="psB")
                            nc.tensor.matmul(psA[:, :N], kblk[0:64, 128 * i:128 * (i + 1)],
                                             qblk[0:64, lo:hi], start=True, stop=True,
                                             tile_position=(0, 0), skip_group_check=True)
                            nc.tensor.matmul(psB[:, :N], kblk[64:128, 128 * i:128 * (i + 1)],
                                             qblk[64:128, lo:hi], start=True, stop=True,
                                             tile_position=(64, 0), skip_group_check=True)
                            sA = stp.tile([128, 512], F32, tag="sA")
                            sB = stp.tile([128, 512], F32, tag="sB")
                            if lo == 128 * i:  # diagonal 128-wide block at the window start
                                nc.vector.tensor_tensor(sA[:, 0:128], psA[:, 0:128],
                                                        mdiag_sb[:], mybir.AluOpType.add)
                                nc.vector.tensor_tensor(sB[:, 0:128], psB[:, 0:128],
                                                        mdiag_sb[:], mybir.AluOpType.add)
                                if N > 128:
                                    nc.vector.tensor_copy(sA[:, 128:N], psA[:, 128:N])
                                    nc.scalar.copy(sB[:, 128:N], psB[:, 128:N])
                            else:
                                if copy_flip & 1:
                                    nc.vector.tensor_copy(sA[:, :N], psA[:, :N])
                                    nc.scalar.copy(sB[:, :N], psB[:, :N])
                                else:
                                    nc.scalar.copy(sA[:, :N], psA[:, :N])
                                    nc.vector.tensor_copy(sB[:, :N], psB[:, :N])
                                copy_flip += 1
                            # 2b: outT[d, q] += v[k, d]^T @ sT[k, q], col-packed heads
                            ol, oh = lo - qbase, hi - qbase
                            last = (i == imax - 1)
                            nc.tensor.matmul(ops[0:64, ol:oh], v_sb[:, i, 128 * p:128 * p + 64],
                                             sA[:, :N], start=(i == 0), stop=last,
                                             tile_position=(0, 0), skip_group_check=True)
                            nc.tensor.matmul(ops[64:128, ol:oh], v_sb[:, i, 128 * p + 64:128 * (p + 1)],
                                             sB[:, :N], start=(i == 0), stop=last,
                                             tile_position=(0, 64), skip_group_check=True)
                    # copy out per 128-q-tile, fusing the -inf suffix contribution
                    # as a per-partition ACT bias (exact IEEE add)
                    for jx in range(8):
                        jj = 8 * qh + jx
                        nc.scalar.add(outT[:, p, qbase + 128 * jx:qbase + 128 * (jx + 1)],
                                      ops[:, 128 * jx:128 * (jx + 1)],
                                      infpT_sb[:, p, jj:jj + 1])

        # ================= Stage F: output projection =================
        # out[t, j] = sum_{512 dcols} outT[d, t] * w2s[d, j]
        with (
            tc.tile_pool(name="fo", bufs=4) as fop,
            tc.tile_pool(name="psf", bufs=4, space="PSUM") as psf,
        ):
            for mt in range(MT_T):
                for nch2 in range(2):
                    ps = psf.tile([128, 512], F32)
                    for kk in range(4):
                        nc.tensor.matmul(ps[:], outT[:, kk, 128 * mt:128 * (mt + 1)],
                                         w2s_sb[:, kk, 512 * nch2:512 * (nch2 + 1)],
                                         start=(kk == 0), stop=(kk == 3))
                    f_sb = fop.tile([128, 512], F32, tag="fsb")
                    if (mt + nch2) & 1:
                        nc.vector.tensor_copy(f_sb[:], ps[:])
                    else:
                        nc.scalar.copy(f_sb[:], ps[:])
                    nc.sync.dma_start(out[128 * mt:128 * (mt + 1), 512 * nch2:512 * (nch2 + 1)],
                                      f_sb[:])


def build_kernel(repeat=1):
    nc = bacc.Bacc(None)
    xt = nc.dram_tensor("xt", [D, T], F32, kind="ExternalInput")
    w1kq = nc.dram_tensor("w1kq", [D, 1024], F32, kind="ExternalInput")
    b1kq = nc.dram_tensor("b1kq", [128, 8], F32, kind="ExternalInput")
    w1v = nc.dram_tensor("w1v", [D, 512], F32, kind="ExternalInput")
    b1v = nc.dram_tensor("b1v", [1, 512], F32, kind="ExternalInput")
    w2s = nc.dram_tensor("w2s", [512, 1024], F32, kind="ExternalInput")
    ut = nc.dram_tensor("ut", [T, 16], F32, kind="ExternalInput")
    mdiag = nc.dram_tensor("mdiag", [128, 128], F32, kind="ExternalInput")
    out = nc.dram_tensor("out", [T, 1024], F32, kind="ExternalOutput")
    tensors = (xt, w1kq, b1kq, w1v, b1v, w2s, ut, mdiag, out)

    with tile.TileContext(nc) as tc:
        if repeat == 1:
            build_body(nc, tc, tensors)
        else:
            with tc.For_i(0, repeat, 1):
                build_body(nc, tc, tensors)
    nc.finalize()
    return nc


def prepare_inputs(x, w1, b1, w2, b2):
    x = np.ascontiguousarray(np.asarray(x, dtype=np.float32))
    w1 = np.asarray(w1, dtype=np.float32)
    b1 = np.asarray(b1, dtype=np.float32)
    w2 = np.asarray(w2, dtype=np.float32)

    jj = np.arange(16)
    kk = np.arange(T)
    ut = np.where(kk[:, None] >= 128 * (jj[None, :] + 1), NEG_INF, np.float32(0.0)).astype(np.float32)
    ki = np.arange(128)
    mdiag = np.where(ki[:, None] > ki[None, :], NEG_INF, np.float32(0.0)).astype(np.float32)

    scale = np.float32(0.125)  # 1/sqrt(64), exact power of two
    in_maps = []
    for c in range(NCORES):
        b, g = divmod(c, 2)
        h0 = 8 * g
        cols = slice(64 * h0, 64 * (h0 + 8))  # 512 cols
        w1k = w1[:, 0 * D:1 * D][:, cols]
        w1q = w1[:, 1 * D:2 * D][:, cols] * scale
        w1v_ = w1[:, 2 * D:3 * D][:, cols]
        b1k = b1[0 * D:1 * D][cols]
        b1q = b1[1 * D:2 * D][cols] * scale
        b1v_ = b1[2 * D:3 * D][cols]
        w1kq = np.ascontiguousarray(np.concatenate([w1k, w1q], axis=1))
        b1kq = np.ascontiguousarray(
            np.concatenate([b1k, b1q]).reshape(8, 128).T)
        in_maps.append({
            "xt": np.ascontiguousarray(x[b].T),
            "w1kq": w1kq,
            "b1kq": b1kq,
            "w1v": np.ascontiguousarray(w1v_),
            "b1v": np.ascontiguousarray(b1v_.reshape(1, 512)),
            "w2s": np.ascontiguousarray(w2[cols, :]),
            "ut": ut,
            "mdiag": mdiag,
        })
    return in_maps


def combine_outputs(results, b2):
    b2 = np.asarray(b2, dtype=np.float32)
    out = np.empty((4, T, D), dtype=np.float32)
    with np.errstate(all="ignore"):
        for b in range(4):
            out[b] = results[2 * b]["out"] + results[2 * b + 1]["out"] + b2
    return out


_NC_CACHE = {}


def kernel(x, w1, b1, w2, b2):
    if "nc" not in _NC_CACHE:
        _NC_CACHE["nc"] = build_kernel(repeat=1)
    nc = _NC_CACHE["nc"]
    in_maps = prepare_inputs(x, w1, b1, w2, b2)
    res = run_bass_kernel_spmd(nc, in_maps, list(range(NCORES)))
    return combine_outputs(res.results, b2)
